# revision 1
# baseline (speedup 1.0000x reference)
"""Trainium2 Bass kernel for nn_Block_65755949302136 (dense transformer block).

Sharding: 8 cores = 2 (batch) x 4 (tensor-parallel ranks). Each rank owns 4
heads (2 sloped-ALiBi + 2 zero-slope, balanced), the matching w_in column
slices (q/k/v/p) and w_out row slice. ReduceScatter(add) over each batch
group after out_proj, LN2 computed locally on each rank's 512-row shard.

v2 dataflow (all feature-major, no on-device transposes of activations):
  - LN1 gamma folded into W host-side; beta rides the ACT bias slots.
  - LN1 stats come out of ap=1 matmuls as token-major COLUMNS (lhsT=x-slice,
    rhs=ones), so the whole mean/var/rstd pipeline runs on [128,16] tiles
    and the stats matmuls are ~free on PE (cost model: ap_size * cycle).
  - x is centered+normalized ONCE into xn (bf16, 2x DVE mode) against
    rstd / (-mu*rstd) broadcasts built by diag(cols) @ all-ones matmuls;
    q/k/p GEMMs read xn -> no extended contraction tile and no per-output
    rstd multiply (their PSUM post-ops ride the ACT bias/scale slots).
  - v GEMM runs on RAW x during the stats+centering window (keeps PE busy);
    the ACT scale-copy retires its psum bank first, then the token-major
    -mu*rstd*colsum(Wv) correction runs as an all-SBUF bf16 DVE stt.
  - Softmax denominator: ap=1 matmuls (free on PE) accumulate per-query
    den columns in PSUM; bf16 column transposes + reciprocal + one
    outer-product matmul rebuild the [dh, q] reciprocal broadcast.
    NOTE matmul start=True resets the WHOLE psum bank -> exactly one
    start per bank, verified on hardware.
  - Slot-0 (steep-slope) heads use a 2-block attention window.
  - Emission is software-pipelined via generators: attn(h) kb-steps are
    interleaved into the qkp(h+1) GEMM phase; out_proj of chunk ch-1 is
    staggered behind attn2/attn3 of chunk ch; residual staging is bf16.
"""

import sys

sys.path.insert(0, "/opt/trn_rl_repo")

import numpy as np

import concourse.bass as bass
import concourse.mybir as mybir
import concourse.tile as tile
from concourse.bass_utils import run_bass_kernel_spmd

F32 = mybir.dt.float32
BF16 = mybir.dt.bfloat16
NP_BF16 = mybir.dt.np(BF16)
AF = mybir.ActivationFunctionType
ALU = mybir.AluOpType

B, L, D, NHEADS, DH = 2, 2048, 1024, 16, 128
DEXP = 2048  # full d_expanded
NH = 4  # heads per core
DL = NH * DH  # 512, local d_expanded slice
KT = D // 128  # 8 k-tiles over d_model
NCH = L // 512  # 4 query chunks
NMT = L // 128  # 16 token tiles
NG = 4  # reduce-scatter groups (512 rows each)

# head assignment: rank r -> [sloped_windowed, sloped_full, zero, zero]
HGROUPS = [[0, 7, 8, 9], [1, 6, 10, 11], [2, 5, 12, 13], [3, 4, 14, 15]]
# per-slot block window (slot0 slopes >= 0.0924: dropped mass < 1e-4 at WB=2)
WB = {0: 2, 1: 16, 2: 16, 3: 16}
NB0 = WB[0]  # slot-0 bias columns in biasv

_CACHED = {}


def _normalize_waits(nc):
    """walrus wait-slot limits are tighter than what Tile emits for some
    instruction classes; move excess sync-waits onto same-engine Drain
    carriers inserted immediately before the instruction."""
    for func in nc.m.functions:
        for blk in func.blocks:
            insts = blk.instructions
            i = 0
            while i < len(insts):
                inst = insts[i]
                si = inst.sync_info
                cap = 1
                if si is not None and len(si.on_wait or []) > cap:
                    waits = list(si.on_wait)
                    excess, keep = waits[:-cap], waits[-cap:]
                    for j, w in enumerate(excess):
                        d = mybir.InstNoOp(
                            name=f"{inst.name}-wsplit{j}",
                            engine=inst.engine,
                            ins=[],
                            outs=[],
                        )
                        d.sync_info = mybir.SyncInfo(on_wait=[w], on_update=[])
                        insts.insert(i, d)
                        nc.register_instruction(d, overwrite=True)
                        i += 1
                    si.on_wait = keep
                i += 1


def build(with_cc=True, b1_zero=True, ln2_trivial=True):
    nc = bass.Bass()

    xt_d = nc.dram_tensor("xt", [D, L], BF16, kind="ExternalInput")
    wq_d = nc.dram_tensor("wq", [D, DL], BF16, kind="ExternalInput")
    wk_d = nc.dram_tensor("wk", [D, DL], BF16, kind="ExternalInput")
    wv_d = nc.dram_tensor("wv", [D, DL], BF16, kind="ExternalInput")
    wp_d = nc.dram_tensor("wp", [D, DL], BF16, kind="ExternalInput")
    wout_d = nc.dram_tensor("wout", [DL, D], BF16, kind="ExternalInput")
    hbc_d = nc.dram_tensor("hbc", [128, 3 * NH], F32, kind="ExternalInput")
    qkb_d = nc.dram_tensor("qkb", [128, 3 * NH], F32, kind="ExternalInput")
    vcb_d = nc.dram_tensor("vcb", [128, DL], BF16, kind="ExternalInput")
    biasv_d = nc.dram_tensor("biasv", [128, NB0 + 19], F32, kind="ExternalInput")
    tri_d = nc.dram_tensor("tri", [128, 128], BF16, kind="ExternalInput")
    idnb_d = nc.dram_tensor("idnb", [128, 128], BF16, kind="ExternalInput")
    if not b1_zero:
        bvbc_d = nc.dram_tensor("bvbc", [128, DL], BF16, kind="ExternalInput")
    if not ln2_trivial:
        g2bc_d = nc.dram_tensor("g2bc", [128, D], F32, kind="ExternalInput")
        b2bc_d = nc.dram_tensor("b2bc", [128, D], F32, kind="ExternalInput")
    out_d = nc.dram_tensor("out", [NG * 128, D], F32, kind="ExternalOutput")

    with tile.TileContext(nc, pool_alloc_mode="queue") as tc:
        cp_cm = tc.tile_pool(name="const", bufs=1)
        cp = cp_cm.__enter__()

        # ---- tiny constants (DMAs are emitted after the x/wv loads: the
        # sync queue is serial and x gates the whole front of the kernel) ----
        tri = cp.tile([128, 128], BF16, tag="tri")
        idnb = cp.tile([128, 128], BF16, tag="idnb")
        hbc = cp.tile([128, 3 * NH], F32, tag="hbc")
        qkb = cp.tile([128, 3 * NH], F32, tag="qkb")
        vcb = cp.tile([128, DL], BF16, tag="vcb")
        biasv = cp.tile([128, NB0 + 19], F32, tag="biasv")

        def emit_const_dmas():
            nc.sync.dma_start(idnb[:], idnb_d[:, :])
            nc.sync.dma_start(vcb[:], vcb_d[:, :])
            nc.sync.dma_start(hbc[:], hbc_d[:, :])
            nc.sync.dma_start(qkb[:], qkb_d[:, :])
            nc.sync.dma_start(tri[:], tri_d[:, :])
            nc.sync.dma_start(biasv[:], biasv_d[:, :])

        ones_bf = cp.tile([128, 1], BF16, tag="ones_bf")
        nc.gpsimd.memset(ones_bf[:], 1.0)
        ones_bfr = cp.tile([1, 128], BF16, tag="ones_bfr")
        nc.gpsimd.memset(ones_bfr[:], 1.0)
        ones128b = cp.tile([128, 128], BF16, tag="ones128b")
        nc.gpsimd.memset(ones128b[:], 1.0)
        eps128 = cp.tile([128, 1], F32, tag="eps128")
        nc.gpsimd.memset(eps128[:], 1e-5)

        inv_bc = [hbc[:, h : h + 1] for h in range(NH)]
        om_bc = [hbc[:, NH + h : NH + h + 1] for h in range(NH)]
        ratio_bc = [hbc[:, 2 * NH + h : 2 * NH + h + 1] for h in range(NH)]
        bqi = [qkb[:, h : h + 1] for h in range(NH)]
        bko = [qkb[:, NH + h : NH + h + 1] for h in range(NH)]
        bp = [qkb[:, 2 * NH + h : 2 * NH + h + 1] for h in range(NH)]
        bias_v = {0: [biasv[:, d : d + 1] for d in range(NB0)]}
        bias_w = [biasv[:, NB0 + i : NB0 + i + 1] for i in range(19)]

        resid_cm = tc.tile_pool(name="resid", bufs=1)
        resid = resid_cm.__enter__()  # geff + vtok
        dram_cm = tc.tile_pool(name="dram", bufs=1, space="DRAM")
        dram = dram_cm.__enter__()
        qkpA_cm = tc.tile_pool(name="qkpA", bufs=1)
        qkpA = qkpA_cm.__enter__()
        rowp_cm = tc.tile_pool(name="rows", bufs=1)
        rowp = rowp_cm.__enter__()  # rs_bc, mrs_bc, nmu/rs cols, row scratch
        xnp_cm = tc.tile_pool(name="xnp", bufs=1)
        xnp = xnp_cm.__enter__()
        wscp_cm = tc.tile_pool(name="wscp", bufs=1)
        wscp = wscp_cm.__enter__()

        xbp_cm = tc.tile_pool(name="xbp", bufs=1)
        xbp = xbp_cm.__enter__()

        xb = []
        wsc = {}
        geff = [resid.tile([128, L], BF16, tag=f"geff{h}", name=f"geff{h}") for h in range(NH)]
        vtok = []
        qT = [qkpA.tile([128, L], BF16, tag=f"qT{h}", name=f"qT{h}") for h in range(NH)]
        kS = [qkpA.tile([128, L], BF16, tag=f"kS{h}", name=f"kS{h}") for h in range(NH)]
        xn = [xnp.tile([128, L], BF16, tag=f"xn{kt}", name=f"xn{kt}") for kt in range(KT)]
        rs_in = [dram.tile([512, D], BF16, tag=f"rsin{g}", name=f"rsin{g}") for g in range(NG)]
        rs_out = [dram.tile([128, D], BF16, tag=f"rsout{g}", name=f"rsout{g}") for g in range(NG)]

        rs_bc = rowp.tile([128, L], BF16, tag="rs_bc")
        mrs_bc = rowp.tile([128, L], BF16, tag="mrs_bc")
        # token-major stat columns [128, 16]
        ncol = rowp.tile([128, NMT], F32, tag="ncol")
        sqm = rowp.tile([128, NMT], F32, tag="sqm")
        rs_cols = rowp.tile([128, NMT], F32, tag="rs_cols")
        nmrsc = rowp.tile([128, NMT], F32, tag="nmrsc")
        nmu_cols = ncol

        if not b1_zero:
            bvbc = cp.tile([128, DL], BF16, tag="bvbc")
            nc.sync.dma_start(bvbc[:], bvbc_d[:, :])

        # ---- stage A: x/wv DMA + column stats (ap=1 matmuls, ~free on PE)
        # + first v-chain group kt-major to fill PE during the DMA window ----
        NG1 = 5
        vmm_cm = tc.tile_pool(name="ps_vm", bufs=6, space="PSUM")
        vmm = vmm_cm.__enter__()
        vps_g1 = [vmm.tile([128, 512], F32, tag="vmm", name=f"vps{m}") for m in range(NG1)]
        with (
            tc.tile_pool(name="ps_sc", bufs=1, space="PSUM") as pscol,
        ):
            stco = pscol.tile([128, 2 * NMT], F32, tag="stco", name="stco")
            scol = stco[:, 0:NMT]
            sqcol = stco[:, NMT : 2 * NMT]
            for kt in range(KT):
                xc = xbp.tile([128, L], BF16, tag=f"xb{kt}", name=f"xb{kt}")
                nc.sync.dma_start(xc[:], xt_d[kt * 128 : (kt + 1) * 128, :])
                xb.append(xc)
                wvt = wscp.tile([128, DL], BF16, tag=f"wv{kt}", name=f"wv{kt}")
                nc.sync.dma_start(wvt[:], wv_d[kt * 128 : (kt + 1) * 128, :])
                wsc.setdefault("v", []).append(wvt)
                xsq = xn[kt]  # xn doubles as the x^2 staging before centering
                if kt % 2 == 0:
                    nc.scalar.activation(xsq[:], xc[:], AF.Square)
                else:  # bf16 TT runs 2x on DVE; keeps ACT off the stats path
                    nc.vector.tensor_mul(xsq[:], xc[:], xc[:])
                # NOTE: start=True resets the WHOLE psum bank, so only the
                # very first matmul touching each bank may carry it.
                for m in range(NMT):
                    msl = slice(m * 128, (m + 1) * 128)
                    nc.tensor.matmul(
                        scol[:, m : m + 1], xc[:, msl], ones_bf[:],
                        start=(kt == 0 and m == 0), stop=(kt == KT - 1),
                        skip_group_check=True,
                    )
                    nc.tensor.matmul(
                        sqcol[:, m : m + 1], xsq[:, msl], ones_bf[:],
                        start=False, stop=(kt == KT - 1),
                        skip_group_check=True,
                    )
                for m in range(NG1):
                    nc.tensor.matmul(
                        vps_g1[m][:], xc[:, m * 128 : (m + 1) * 128], wvt[:],
                        start=(kt == 0), stop=(kt == KT - 1),
                    )
            emit_const_dmas()
            for kind, wd in (("q", wq_d), ("k", wk_d), ("p", wp_d)):
                tiles = []
                for kt in range(KT):
                    t = wscp.tile([128, DL], BF16, tag=f"w{kind}{kt}", name=f"w{kind}{kt}")
                    nc.sync.dma_start(t[:], wd[kt * 128 : (kt + 1) * 128, :])
                    tiles.append(t)
                wsc[kind] = tiles

            # column-space LN1 stats: all ops on [128, 16]
            nc.vector.tensor_scalar_mul(ncol[:], scol[:], -1.0 / D)
            nc.vector.tensor_scalar_mul(sqm[:], sqcol[:], 1.0 / D)
            nc.vector.tensor_mul(nmrsc[:], ncol[:], ncol[:])
            nc.vector.tensor_sub(sqm[:], sqm[:], nmrsc[:])  # var
            nc.scalar.activation(sqm[:], sqm[:], AF.Sqrt, bias=eps128[:])
            nc.vector.reciprocal(rs_cols[:], sqm[:])
            nc.vector.tensor_mul(nmrsc[:], ncol[:], rs_cols[:])

        # ---- stage B: broadcasts, remaining v chains, centering ----
        def emit_vpost(m, vps):
            # scale-copy first (frees the psum bank), then the token-major
            # -mu*rstd*colsum(Wv) correction runs all-SBUF on the idle Pool
            vt = resid.tile([128, DL], BF16, tag=f"vtok{m}", name=f"vtok{m}")
            nc.scalar.activation(vt[:], vps[:], AF.Copy, scale=rs_cols[:, m : m + 1])
            with nc.allow_low_precision("bf16 smear-v correction"):
                nc.vector.scalar_tensor_tensor(
                    vt[:], vcb[:], nmrsc[:, m : m + 1], vt[:], ALU.mult, ALU.add
                )
                if not b1_zero:  # beta term applied post-scale: * rstd_tok
                    nc.vector.scalar_tensor_tensor(
                        vt[:], bvbc[:], rs_cols[:, m : m + 1], vt[:],
                        ALU.mult, ALU.add,
                    )
            vtok.append(vt)

        def emit_vchain(m):
            msl = slice(m * 128, (m + 1) * 128)
            vps = vmm.tile([128, 512], F32, tag="vmm", name=f"vps{m}")
            for kt in range(KT):
                nc.tensor.matmul(vps[:], xb[kt][:, msl], wsc["v"][kt][:],
                                 start=(kt == 0), stop=(kt == KT - 1))
            return vps

        def emit_centering(ch, eng):
            # xn = x * rs_bc, then += mrs_bc in place (bf16 SBUF, 2x on DVE)
            csl = slice(ch * 512, (ch + 1) * 512)
            for kt in range(KT):
                eng.tensor_mul(xn[kt][:, csl], xb[kt][:, csl], rs_bc[:, csl])
                eng.tensor_add(xn[kt][:, csl], xn[kt][:, csl], mrs_bc[:, csl])

        # m5 takes the spare psum slot with no wait; vposts free G1 slots
        vps_pend = [(NG1, emit_vchain(NG1))]
        for m in range(3):
            emit_vpost(m, vps_g1[m])
        vps_pend.append((NG1 + 1, emit_vchain(NG1 + 1)))
        vps_pend.append((NG1 + 2, emit_vchain(NG1 + 2)))

        # rstd / (-mu*rstd) broadcasts: diag(cols) matmul against all-ones;
        # diag construction rides the idle Pool engine
        with (
            tc.tile_pool(name="pbc", bufs=2, space="PSUM") as pbc,
            tc.tile_pool(name="dgp", bufs=8) as dgp,
        ):
            for ch in range(NCH):
                sl = slice(ch * 512, (ch + 1) * 512)
                for si, (src, dst) in enumerate(((rs_cols, rs_bc), (nmrsc, mrs_bc))):
                    bps = pbc.tile([128, 512], F32, tag="bcps", name=f"bc{si}_{ch}")
                    for mi in range(4):
                        m = 4 * ch + mi
                        dg = dgp.tile([128, 128], BF16, tag="dg")
                        nc.vector.tensor_scalar_mul(dg[:], idnb[:], src[:, m : m + 1])
                        nc.tensor.matmul(bps[:, mi * 128 : (mi + 1) * 128],
                                         ones128b[:], dg[:], start=(mi == 0),
                                         stop=(mi == 3), skip_group_check=True)
                    nc.scalar.copy(dst[:, sl], bps[:])

        for m in range(3, NG1):
            emit_vpost(m, vps_g1[m])
        emit_centering(3, nc.gpsimd)  # Pool trails; qkp reaches ch3 last
        for m in range(NG1 + 3, NMT):
            vps_pend.append((m, emit_vchain(m)))
            if len(vps_pend) >= 3:
                emit_vpost(*vps_pend.pop(0))
        while vps_pend:
            emit_vpost(*vps_pend.pop(0))
        for ch in range(3):
            emit_centering(ch, nc.vector)

        vmm_cm.__exit__(None, None, None)
        xbp_cm.__exit__(None, None, None)  # raw x no longer needed
        pmm_cm = tc.tile_pool(name="ps_mm", bufs=3, space="PSUM")
        pmm = pmm_cm.__enter__()

        wop_cm = tc.tile_pool(name="wo", bufs=1)
        wop = wop_cm.__enter__()
        woutT = []
        for h in range(NH):
            t = wop.tile([128, D], BF16, tag=f"woutT{h}", name=f"woutT{h}")
            woutT.append(t)
        if not ln2_trivial:
            g2bc = wop.tile([128, D], F32, tag="g2bc")
            b2bc = wop.tile([128, D], F32, tag="b2bc")

        # ---- stages C-E: head-pipelined qkp + attention; out_proj rides h3 ----
        kkp_cm = tc.tile_pool(name="kk", bufs=1)
        kkp = kkp_cm.__enter__()
        with (
            tc.tile_pool(name="ps_s", bufs=2, space="PSUM") as pss,
            tc.tile_pool(name="ps_o", bufs=2, space="PSUM") as pso,
            tc.tile_pool(name="ps_den", bufs=1, space="PSUM") as psd,
            tc.tile_pool(name="et", bufs=16) as etp,
            tc.tile_pool(name="dn", bufs=4) as dnp,
            tc.tile_pool(name="ln2", bufs=2) as lnp,
            tc.tile_pool(name="ostage", bufs=4) as osp,
        ):

            def gen_qkp(h):
                hsl = slice(h * 128, (h + 1) * 128)
                kk = kkp.tile([128, L], BF16, tag="kk", name=f"kk{h}")
                for ch in range(NCH):
                    csl = slice(ch * 512, (ch + 1) * 512)
                    qps = pmm.tile([128, 512], F32, tag="mm", name=f"qps{h}_{ch}")
                    for kt in range(KT):
                        nc.tensor.matmul(qps[:], wsc["q"][kt][:, hsl], xn[kt][:, csl],
                                         start=(kt == 0), stop=(kt == KT - 1))
                    nc.scalar.activation(qT[h][:, csl], qps[:], AF.Identity,
                                         bias=bqi[h], scale=inv_bc[h])
                    yield

                    kps = pmm.tile([128, 512], F32, tag="mm", name=f"kps{h}_{ch}")
                    for kt in range(KT):
                        nc.tensor.matmul(kps[:], wsc["k"][kt][:, hsl], xn[kt][:, csl],
                                         start=(kt == 0), stop=(kt == KT - 1))
                    nc.scalar.activation(kk[:, csl], kps[:], AF.Identity,
                                         bias=bko[h], scale=om_bc[h])
                    yield

                    pps = pmm.tile([128, 512], F32, tag="mm", name=f"pps{h}_{ch}")
                    for kt in range(KT):
                        nc.tensor.matmul(pps[:], wsc["p"][kt][:, hsl], xn[kt][:, csl],
                                         start=(kt == 0), stop=(kt == KT - 1))
                    nc.scalar.activation(geff[h][:, csl], pps[:], AF.Silu, bias=bp[h])
                    # smear per-chunk so attention can start behind the k ACTs
                    cs, ce = ch * 512, (ch + 1) * 512
                    if ch == 0:
                        nc.vector.tensor_copy(kS[h][:, 0:1], kk[:, 0:1])
                        nc.vector.scalar_tensor_tensor(
                            kS[h][:, 1:512], kk[:, 0:511], ratio_bc[h],
                            kk[:, 1:512], ALU.mult, ALU.add,
                        )
                    else:
                        nc.vector.scalar_tensor_tensor(
                            kS[h][:, cs:ce], kk[:, cs - 1 : ce - 1], ratio_bc[h],
                            kk[:, cs:ce], ALU.mult, ALU.add,
                        )
                    yield

            def gen_attn(h, chs=None):
                hsl = slice(h * 128, (h + 1) * 128)
                for ch in (range(NCH) if chs is None else chs):
                    csl = slice(ch * 512, (ch + 1) * 512)
                    kb_lo = max(0, 4 * ch + 1 - WB[h])
                    kb_hi = 4 * ch + 3
                    ops_ps = pso.tile([128, 512], F32, tag="ops", name=f"ops{h}_{ch}")
                    dsc = psd.tile([128, 512], F32, tag="den", name=f"den{h}_{ch}")
                    den_ps = dsc[:, 0:4]
                    first = {qs: None for qs in range(4)}
                    state = {"den_started": False}

                    def emit_ops(kb, qs0, qs1, esl, et):
                        st = all(first[qs] is None for qs in range(qs0, qs1))
                        for qs in range(qs0, qs1):
                            if first[qs] is None:
                                first[qs] = kb
                        nc.tensor.matmul(
                            ops_ps[:, esl], vtok[kb][:, hsl], et[:, esl],
                            start=st, stop=(kb == kb_hi),
                        )
                        for qs in range(qs0, qs1):
                            qsl = slice(qs * 128, (qs + 1) * 128)
                            nc.tensor.matmul(
                                den_ps[:, qs : qs + 1], et[:, qsl], ones_bf[:],
                                start=(not state["den_started"]),
                                stop=(kb == 4 * ch + qs),
                                skip_group_check=True,
                            )
                            state["den_started"] = True

                    # software-pipeline by two kb: S(k+2)/exp(k+2) issue before
                    # ops(k), so PE never blocks in-order on exp latencies
                    pend_ops = []
                    for kb in range(kb_lo, kb_hi + 1):
                        qs0 = max(0, kb - 4 * ch)
                        qs1 = min(4, kb - 4 * ch + WB[h])
                        if qs0 >= qs1:
                            continue
                        nsl = slice(csl.start + qs0 * 128, csl.start + qs1 * 128)
                        esl = slice(qs0 * 128, qs1 * 128)
                        sps = pss.tile([128, 512], F32, tag="sps", name=f"sps{h}_{ch}_{kb}")
                        nc.tensor.matmul(
                            sps[:, esl], kS[h][:, kb * 128 : (kb + 1) * 128],
                            qT[h][:, nsl], start=True, stop=True,
                        )
                        et = etp.tile([128, 512], BF16, tag="et")
                        if h == 0:
                            for qs in range(qs0, qs1):
                                qsl = slice(qs * 128, (qs + 1) * 128)
                                dd = (4 * ch + qs) - kb
                                nc.scalar.activation(
                                    et[:, qsl], sps[:, qsl], AF.Exp,
                                    bias=bias_v[h][dd],
                                )
                        elif h == 1:
                            nc.scalar.activation(
                                et[:, esl], sps[:, esl], AF.Exp,
                                bias=bias_w[4 * ch - kb + 3],
                            )
                        else:
                            nc.scalar.activation(et[:, esl], sps[:, esl], AF.Exp)
                        for qs in range(qs0, qs1):
                            if (4 * ch + qs) == kb:
                                qsl = slice(qs * 128, (qs + 1) * 128)
                                nc.vector.tensor_mul(et[:, qsl], et[:, qsl], tri[:])
                        yield
                        if len(pend_ops) >= 2:
                            emit_ops(*pend_ops.pop(0))
                        pend_ops.append((kb, qs0, qs1, esl, et))
                        yield
                    for p in pend_ops:
                        emit_ops(*p)
                    den_sb = dnp.tile([128, 4], BF16, tag="densb")
                    with nc.allow_low_precision("bf16 den feeds transpose"):
                        nc.vector.tensor_copy(den_sb[:], den_ps[:])
                    dr_ps = dsc[0:1, 64:320].bitcast(BF16)
                    for qs in range(4):
                        nc.tensor.matmul(
                            dr_ps[0:1, qs * 128 : (qs + 1) * 128],
                            den_sb[:, qs : qs + 1], idnb[:],
                            start=(qs == 0), stop=(qs == 3), is_transpose=True,
                            skip_group_check=True,
                        )
                    dinv = dnp.tile([1, 512], BF16, tag="dinv")
                    with nc.allow_low_precision("bf16 1/den feeds a bf16 matmul"):
                        nc.vector.reciprocal(dinv[:], dr_ps[:])
                    # dbc borrows the pso ring (ops slot of two units ago is free)
                    dbc_ps = pso.tile([128, 512], F32, tag="ops", name=f"dbc{h}_{ch}")
                    nc.tensor.matmul(dbc_ps[:], ones_bfr[:], dinv[:],
                                     start=True, stop=True)
                    dbc = dnp.tile([128, 512], BF16, tag="dbc")
                    with nc.allow_low_precision("bf16 1/den broadcast"):
                        nc.vector.tensor_copy(dbc[:], dbc_ps[:])
                    ozc = dnp.tile([128, 512], BF16, tag="ozc")
                    nc.vector.tensor_mul(ozc[:], ops_ps[:], dbc[:])
                    nc.vector.tensor_mul(geff[h][:, csl], ozc[:], geff[h][:, csl])
                    yield

            def emit_attn(h, chs=None):
                for _ in gen_attn(h, chs):
                    pass

            def interleave(main, bg, k):
                # advance bg ~k steps per main step; exhaust both
                carry = 0.0
                done = False
                for _ in main:
                    carry += k
                    while carry >= 1.0 and not done:
                        carry -= 1.0
                        try:
                            next(bg)
                        except StopIteration:
                            done = True
                while not done:
                    try:
                        next(bg)
                    except StopIteration:
                        done = True

            def outproj_chain(g, mi, nch2, eng_act):
                def emit():
                    m = 4 * g + mi
                    msl = slice(m * 128, (m + 1) * 128)
                    nsl2 = slice(nch2 * 512, (nch2 + 1) * 512)
                    op2 = pmm.tile([128, 512], F32, tag="mm", name=f"op2_{m}_{nch2}")
                    for hh in range(NH):
                        nc.tensor.matmul(
                            op2[:], geff[hh][:, msl], woutT[hh][:, nsl2],
                            start=(hh == 0), stop=(hh == NH - 1),
                        )
                    osb = osp.tile([128, 512], BF16, tag="osb")
                    with nc.allow_low_precision("bf16 residual staging"):
                        if eng_act:
                            nc.scalar.copy(osb[:], op2[:])
                        else:
                            nc.vector.tensor_copy(osb[:], op2[:])
                    nc.sync.dma_start(
                        rs_in[g][mi * 128 : (mi + 1) * 128, nsl2], osb[:]
                    )

                return emit

            def outproj_closures(g, act_frac=2):
                # act_frac: every act_frac-th copy goes to ACT (rest DVE);
                # the last groups run after the exps drain, so ACT is free
                return [
                    outproj_chain(g, mi, nch2,
                                  eng_act=((mi + nch2) % 2 == 0))
                    for mi in range(4)
                    for nch2 in range(2)
                ]

            def emit_outproj_fin(g):
                    if True:
                        if with_cc:
                            nc.gpsimd.collective_compute(
                                "ReduceScatter", ALU.add,
                                replica_groups=[[0, 1, 2, 3], [4, 5, 6, 7]],
                                ins=[rs_in[g][:, :].opt()],
                                outs=[rs_out[g][:, :].opt()],
                            )
                        else:
                            nc.sync.dma_start(rs_out[g][:, :], rs_in[g][0:128, :])
                        yt = lnp.tile([128, D], BF16, tag="yt")
                        nc.sync.dma_start(yt[:], rs_out[g][:, :])
                        bs = lnp.tile([128, 12], F32, tag="bs")
                        nc.vector.bn_stats(bs[:, 0:6], yt[:, 0:512])
                        nc.vector.bn_stats(bs[:, 6:12], yt[:, 512:1024])
                        ag = lnp.tile([128, 2], F32, tag="ag")
                        nc.vector.bn_aggr(ag[:], bs[:])
                        sd2 = lnp.tile([128, 1], F32, tag="sd2")
                        nc.scalar.activation(sd2[:], ag[:, 1:2], AF.Sqrt, bias=eps128[:])
                        rstd2 = lnp.tile([128, 1], F32, tag="rstd2")
                        nc.vector.reciprocal(rstd2[:], sd2[:])
                        nmu2 = lnp.tile([128, 1], F32, tag="nmu2")
                        nc.vector.scalar_tensor_tensor(
                            nmu2[:], ag[:, 0:1], -1.0, rstd2[:], ALU.mult, ALU.mult
                        )
                        t2 = lnp.tile([128, D], F32, tag="t2")
                        nc.scalar.activation(t2[:], yt[:], AF.Identity, bias=nmu2[:], scale=rstd2[:])
                        if ln2_trivial:
                            nc.sync.dma_start(out_d[g * 128 : (g + 1) * 128, :], t2[:])
                        else:
                            t3 = lnp.tile([128, D], F32, tag="t3")
                            nc.vector.tensor_mul(t3[:], t2[:], g2bc[:])
                            nc.vector.tensor_add(t3[:], t3[:], b2bc[:])
                            nc.sync.dma_start(out_d[g * 128 : (g + 1) * 128, :], t3[:])

            # software-pipelined emission: attention(h) kb-steps ride inside
            # the qkp(h+1) GEMM phase so exps hide under dense matmul cover
            for _ in gen_qkp(0):
                pass
            interleave(gen_qkp(1), gen_attn(0), k=6.4)
            interleave(gen_qkp(2), gen_attn(1), k=9.0)
            for h in range(NH):
                nc.sync.dma_start(woutT[h][:], wout_d[h * 128 : (h + 1) * 128, :])
            if not ln2_trivial:
                nc.sync.dma_start(g2bc[:], g2bc_d[:, :])
                nc.sync.dma_start(b2bc[:], b2bc_d[:, :])
            for _ in gen_qkp(3):
                pass
            # stagger: attn2/attn3 of the next chunk cover the latency of
            # chunk ch's geff[3] chain before its out_proj consumes it
            emit_attn(2, [0])
            emit_attn(3, [0])
            for ch in range(1, NCH):
                emit_attn(2, [ch])
                emit_attn(3, [ch])
                for c in outproj_closures(ch - 1):
                    c()
                emit_outproj_fin(ch - 1)
            for c in outproj_closures(NCH - 1):
                c()
            emit_outproj_fin(NCH - 1)

        kkp_cm.__exit__(None, None, None)
        wop_cm.__exit__(None, None, None)
        pmm_cm.__exit__(None, None, None)
        wscp_cm.__exit__(None, None, None)
        xnp_cm.__exit__(None, None, None)
        rowp_cm.__exit__(None, None, None)
        qkpA_cm.__exit__(None, None, None)
        dram_cm.__exit__(None, None, None)
        resid_cm.__exit__(None, None, None)
        cp_cm.__exit__(None, None, None)

    _normalize_waits(nc)
    return nc


def _slopes16():
    half = NHEADS // 2
    return np.concatenate(
        [2.0 ** np.linspace(0.0, -8.0, half), np.zeros(NHEADS - half)]
    ).astype(np.float32)


def kernel(x, ln1_g, ln1_b, ln2_g, ln2_b, w_in, w_out, smear_factor, log_scale):
    x = np.asarray(x, np.float32)
    w_in = np.asarray(w_in, np.float32)
    w_out = np.asarray(w_out, np.float32)
    ln1_g = np.asarray(ln1_g, np.float32)
    ln1_b = np.asarray(ln1_b, np.float32)
    ln2_g = np.asarray(ln2_g, np.float32)
    ln2_b = np.asarray(ln2_b, np.float32)
    smear_factor = np.asarray(smear_factor, np.float32)
    log_scale = np.asarray(log_scale, np.float32)

    b1_zero = not np.any(ln1_b)
    ln2_trivial = (not np.any(ln2_b)) and np.all(ln2_g == 1.0)
    key = ("nc", b1_zero, ln2_trivial)
    if key not in _CACHED:
        _CACHED[key] = build(b1_zero=b1_zero, ln2_trivial=ln2_trivial)
    nc = _CACHED[key]

    # fold ln1 gamma into w_in host-side
    wg = w_in * ln1_g[:, None]
    bw = ln1_b @ wg  # [4*DEXP] contribution of ln1 beta

    slopes16 = _slopes16()
    jj = np.arange(128)
    tri = (jj[:, None] <= jj[None, :]).astype(NP_BF16)  # keep j <= i
    idn = np.eye(128, dtype=np.float32)  # bf16 identity for PE transposes

    in_maps = []
    for c in range(8):
        b, r = divmod(c, 4)
        hs = HGROUPS[r]
        cols = np.concatenate([np.arange(h * 128, (h + 1) * 128) for h in hs])
        sl = slopes16[hs]
        inv = np.exp(-2.0 * log_scale[hs]) / np.sqrt(128.0)
        sg = 1.0 / (1.0 + np.exp(-smear_factor[hs]))
        om = 1.0 - sg
        ratio = np.exp(smear_factor[hs])
        hbc = np.tile(
            np.concatenate([inv, om, ratio]).reshape(1, 3 * NH), (128, 1)
        ).astype(np.float32)
        # per-head ln1-beta bias columns: q scaled by inv, k by om, p raw
        bq = bw[0 * DEXP + cols].reshape(NH, 128)
        bk = bw[1 * DEXP + cols].reshape(NH, 128)
        bpv = bw[3 * DEXP + cols].reshape(NH, 128)
        qkb = np.concatenate(
            [bq.T * inv[None, :], bk.T * om[None, :], bpv.T], axis=1
        ).astype(np.float32)  # [128, 12]
        wv_sl = np.ascontiguousarray(wg[:, 2 * DEXP + cols]).astype(np.float32)
        vcb = np.tile(wv_sl.sum(axis=0, dtype=np.float64).astype(np.float32)[None, :], (128, 1))
        iota_c = np.arange(128, dtype=np.float32)
        bias_cols = [sl[0] * (iota_c - 128 * d - 63) for d in range(NB0)]
        # slot1: one vector per dd = 4*ch - kb in [-3, 15]:
        bias_cols += [sl[1] * (iota_c - 128 * d - 447) for d in range(-3, 16)]
        biasv = np.stack(bias_cols, axis=1).astype(np.float32)
        m = {
            "xt": np.ascontiguousarray(x[b].T).astype(NP_BF16),
            "wq": np.ascontiguousarray(wg[:, 0 * DEXP + cols]).astype(NP_BF16),
            "wk": np.ascontiguousarray(wg[:, 1 * DEXP + cols]).astype(NP_BF16),
            "wv": wv_sl.astype(NP_BF16),
            "wp": np.ascontiguousarray(wg[:, 3 * DEXP + cols]).astype(NP_BF16),
            "wout": np.ascontiguousarray(w_out[cols, :]).astype(NP_BF16),
            "hbc": hbc,
            "qkb": qkb,
            "vcb": vcb.astype(NP_BF16),
            "biasv": biasv,
            "tri": tri,
            "idnb": idn.astype(NP_BF16),
        }
        if not b1_zero:
            m["bvbc"] = np.tile(bw[2 * DEXP + cols][None, :], (128, 1)).astype(NP_BF16)
        if not ln2_trivial:
            m["g2bc"] = np.tile(ln2_g.reshape(1, D), (128, 1)).astype(np.float32)
            m["b2bc"] = np.tile(ln2_b.reshape(1, D), (128, 1)).astype(np.float32)
        in_maps.append(m)

    res = None
    last_exc = None
    for _attempt in range(3):
        try:
            res = run_bass_kernel_spmd(nc, in_maps, core_ids=list(range(8)))
            break
        except Exception as e:  # transient axon worker drops; retry
            last_exc = e
            import time as _time

            _time.sleep(2.0)
    if res is None:
        raise last_exc
    _CACHED["last_res"] = res
    out = np.empty((B, L, D), np.float32)
    for c in range(8):
        b, r = divmod(c, 4)
        o = res.results[c]["out"]  # [512, 1024]
        for g in range(NG):
            out[b, 512 * g + 128 * r : 512 * g + 128 * r + 128, :] = o[
                128 * g : 128 * (g + 1), :
            ]
    return out



# revision 54
# speedup vs baseline: 1.0302x; 1.0302x over previous
"""Trainium2 Bass kernel for nn_Block_65755949302136 (dense transformer block).

Sharding: 8 cores = 2 (batch) x 4 (tensor-parallel ranks). Each rank owns 4
heads (2 sloped-ALiBi + 2 zero-slope, balanced), the matching w_in column
slices (q/k/v/p) and w_out row slice. ReduceScatter(add) over each batch
group after out_proj, LN2 computed locally on each rank's 512-row shard.

v2 dataflow (all feature-major, no on-device transposes of activations):
  - LN1 gamma folded into W host-side; beta rides the ACT bias slots.
  - LN1 stats come out of ap=1 matmuls as token-major COLUMNS (lhsT=x-slice,
    rhs=ones), so the whole mean/var/rstd pipeline runs on [128,16] tiles
    and the stats matmuls are ~free on PE (cost model: ap_size * cycle).
  - x is centered+normalized ONCE into xn (bf16, 2x DVE mode) against
    rstd / (-mu*rstd) broadcasts built by diag(cols) @ all-ones matmuls;
    q/k/p GEMMs read xn -> no extended contraction tile and no per-output
    rstd multiply (their PSUM post-ops ride the ACT bias/scale slots).
  - v GEMM runs on RAW x during the stats+centering window (keeps PE busy);
    the ACT scale-copy retires its psum bank first, then the token-major
    -mu*rstd*colsum(Wv) correction runs as an all-SBUF bf16 DVE stt.
  - Softmax denominator: ap=1 matmuls (free on PE) accumulate per-query
    den columns in PSUM; bf16 column transposes + reciprocal + one
    outer-product matmul rebuild the [dh, q] reciprocal broadcast.
    NOTE matmul start=True resets the WHOLE psum bank -> exactly one
    start per bank, verified on hardware.
  - Slot-0 (steep-slope) heads use a 2-block attention window.
  - Emission is software-pipelined via generators: attn(h) kb-steps are
    interleaved into the qkp(h+1) GEMM phase; out_proj of chunk ch-1 is
    staggered behind attn2/attn3 of chunk ch; residual staging is bf16.
"""

import sys

sys.path.insert(0, "/opt/trn_rl_repo")

import numpy as np

import concourse.bass as bass
import concourse.mybir as mybir
import concourse.tile as tile
from concourse.bass_utils import run_bass_kernel_spmd

F32 = mybir.dt.float32
BF16 = mybir.dt.bfloat16
NP_BF16 = mybir.dt.np(BF16)
AF = mybir.ActivationFunctionType
ALU = mybir.AluOpType

B, L, D, NHEADS, DH = 2, 2048, 1024, 16, 128
DEXP = 2048  # full d_expanded
NH = 4  # heads per core
DL = NH * DH  # 512, local d_expanded slice
KT = D // 128  # 8 k-tiles over d_model
NCH = L // 512  # 4 query chunks
NMT = L // 128  # 16 token tiles
NG = 4  # reduce-scatter groups (512 rows each)
E4 = mybir.dt.float8e4
NP_E4 = mybir.dt.np(E4)
PM = mybir.MatmulPerfMode

# head assignment: rank r -> [sloped_windowed, sloped_full, zero, zero]
HGROUPS = [[0, 7, 8, 9], [1, 6, 10, 11], [2, 5, 12, 13], [3, 4, 14, 15]]
# per-slot block window (slot0 slopes >= 0.0924: dropped mass < 1e-4 at WB=2)
WB = {0: 2, 1: 16, 2: 16, 3: 16}
NB0 = WB[0]  # slot-0 bias columns in biasv

_CACHED = {}


def _normalize_waits(nc):
    """walrus wait-slot limits are tighter than what Tile emits for some
    instruction classes; move excess sync-waits onto same-engine Drain
    carriers inserted immediately before the instruction."""
    for func in nc.m.functions:
        for blk in func.blocks:
            insts = blk.instructions
            i = 0
            while i < len(insts):
                inst = insts[i]
                si = inst.sync_info
                cap = 1
                if si is not None and len(si.on_wait or []) > cap:
                    waits = list(si.on_wait)
                    excess, keep = waits[:-cap], waits[-cap:]
                    for j, w in enumerate(excess):
                        d = mybir.InstNoOp(
                            name=f"{inst.name}-wsplit{j}",
                            engine=inst.engine,
                            ins=[],
                            outs=[],
                        )
                        d.sync_info = mybir.SyncInfo(on_wait=[w], on_update=[])
                        insts.insert(i, d)
                        nc.register_instruction(d, overwrite=True)
                        i += 1
                    si.on_wait = keep
                i += 1


def build(with_cc=True, b1_zero=True, ln2_trivial=True, swp=10):
    nc = bass.Bass()

    xt_d = nc.dram_tensor("xt", [D, L], BF16, kind="ExternalInput")
    # q/k/p weights ship as fp8 e4m3 hi+lo pairs packed [128, KT, 2, DL]
    # (plane0=hi, plane1=lo), pre-scaled by 2**sw host-side; the descale
    # rides the ACT post-op scale slots (hbc for q/k, float const for p).
    wq_d = nc.dram_tensor("wq", [128, KT * 2 * DL], E4, kind="ExternalInput")
    wk_d = nc.dram_tensor("wk", [128, KT * 2 * DL], E4, kind="ExternalInput")
    wv_d = nc.dram_tensor("wv", [D, DL], BF16, kind="ExternalInput")
    wp_d = nc.dram_tensor("wp", [128, KT * 2 * DL], E4, kind="ExternalInput")
    wout_d = nc.dram_tensor("wout", [DL, D], BF16, kind="ExternalInput")
    hbc_d = nc.dram_tensor("hbc", [128, 3 * NH], F32, kind="ExternalInput")
    qkb_d = nc.dram_tensor("qkb", [128, 3 * NH], F32, kind="ExternalInput")
    vcb_d = nc.dram_tensor("vcb", [128, DL], BF16, kind="ExternalInput")
    biasv_d = nc.dram_tensor("biasv", [128, NB0 + 19], F32, kind="ExternalInput")
    tri_d = nc.dram_tensor("tri", [128, 128], BF16, kind="ExternalInput")
    idnb_d = nc.dram_tensor("idnb", [128, 128], BF16, kind="ExternalInput")
    if not b1_zero:
        bvbc_d = nc.dram_tensor("bvbc", [128, DL], BF16, kind="ExternalInput")
    if not ln2_trivial:
        g2bc_d = nc.dram_tensor("g2bc", [128, D], F32, kind="ExternalInput")
        b2bc_d = nc.dram_tensor("b2bc", [128, D], F32, kind="ExternalInput")
    out_d = nc.dram_tensor("out", [NG * 128, D], BF16, kind="ExternalOutput")

    with tile.TileContext(nc, pool_alloc_mode="queue") as tc:
        cp_cm = tc.tile_pool(name="const", bufs=1)
        cp = cp_cm.__enter__()

        # ---- tiny constants (DMAs are emitted after the x/wv loads: the
        # sync queue is serial and x gates the whole front of the kernel) ----
        tri = cp.tile([128, 128], BF16, tag="tri")
        idnb = cp.tile([128, 128], BF16, tag="idnb")
        hbc = cp.tile([128, 3 * NH], F32, tag="hbc")
        qkb = cp.tile([128, 3 * NH], F32, tag="qkb")
        vcb = cp.tile([128, DL], BF16, tag="vcb")
        biasv = cp.tile([128, NB0 + 19], F32, tag="biasv")

        def emit_const_dmas():
            nc.sync.dma_start(idnb[:], idnb_d[:, :])
            nc.sync.dma_start(vcb[:], vcb_d[:, :])
            nc.sync.dma_start(hbc[:], hbc_d[:, :])
            nc.sync.dma_start(qkb[:], qkb_d[:, :])
            nc.sync.dma_start(tri[:], tri_d[:, :])
            nc.sync.dma_start(biasv[:], biasv_d[:, :])

        ones_bf = cp.tile([128, 1], BF16, tag="ones_bf")
        nc.gpsimd.memset(ones_bf[:], 1.0)
        ones_bfr = cp.tile([1, 128], BF16, tag="ones_bfr")
        nc.gpsimd.memset(ones_bfr[:], 1.0)
        ones128b = cp.tile([128, 128], BF16, tag="ones128b")
        nc.gpsimd.memset(ones128b[:], 1.0)
        eps128 = cp.tile([128, 1], F32, tag="eps128")
        nc.gpsimd.memset(eps128[:], 1e-5)

        inv_bc = [hbc[:, h : h + 1] for h in range(NH)]
        om_bc = [hbc[:, NH + h : NH + h + 1] for h in range(NH)]
        ratio_bc = [hbc[:, 2 * NH + h : 2 * NH + h + 1] for h in range(NH)]
        bqi = [qkb[:, h : h + 1] for h in range(NH)]
        bko = [qkb[:, NH + h : NH + h + 1] for h in range(NH)]
        bp = [qkb[:, 2 * NH + h : 2 * NH + h + 1] for h in range(NH)]
        bias_v = {0: [biasv[:, d : d + 1] for d in range(NB0)]}
        bias_w = [biasv[:, NB0 + i : NB0 + i + 1] for i in range(19)]

        resid_cm = tc.tile_pool(name="resid", bufs=1)
        resid = resid_cm.__enter__()  # geff + vtok
        dram_cm = tc.tile_pool(name="dram", bufs=1, space="DRAM")
        dram = dram_cm.__enter__()
        qkpA_cm = tc.tile_pool(name="qkpA", bufs=1)
        qkpA = qkpA_cm.__enter__()
        rowp_cm = tc.tile_pool(name="rows", bufs=1)
        rowp = rowp_cm.__enter__()  # rs_bc, mrs_bc, nmu/rs cols, row scratch
        xnp_cm = tc.tile_pool(name="xnp", bufs=1)
        xnp = xnp_cm.__enter__()
        wscp_cm = tc.tile_pool(name="wscp", bufs=1)
        wscp = wscp_cm.__enter__()
        # wop/kkp sit under xbp in the pool stack so xbp+xnr can pop before
        # the attention scratch pools open (their space is reused for et/dn)
        wop_cm = tc.tile_pool(name="wo", bufs=1)
        wop = wop_cm.__enter__()
        kkp_cm = tc.tile_pool(name="kk", bufs=1)
        kkp = kkp_cm.__enter__()

        xbp_cm = tc.tile_pool(name="xbp", bufs=1)
        xbp = xbp_cm.__enter__()

        xb = []
        wsc = {}
        geff = [resid.tile([128, L], BF16, tag=f"geff{h}", name=f"geff{h}") for h in range(NH)]
        vtok = []
        qT = [qkpA.tile([128, L], BF16, tag=f"qT{h}", name=f"qT{h}") for h in range(NH)]
        kS = [qkpA.tile([128, L], BF16, tag=f"kS{h}", name=f"kS{h}") for h in range(NH)]
        # fp8 hi/lo planes of the centered-normalized x, packed [128, KT, 2, L]
        # with plane0=lo, plane1=hi (pairs with W's plane0=hi, plane1=lo so the
        # cross-term DoubleRow reads (Whi,Wlo)x(xlo,xhi) with natural strides)
        xnq = xnp.tile([128, KT, 2, L], E4, tag="xnq", name="xnq")
        xnr_cm = tc.tile_pool(name="xnr", bufs=3)
        xnr = xnr_cm.__enter__()
        rs_in = [dram.tile([512, D], BF16, tag=f"rsin{g}", name=f"rsin{g}") for g in range(NG)]
        rs_out = [dram.tile([128, D], BF16, tag=f"rsout{g}", name=f"rsout{g}") for g in range(NG)]

        rs_bc = rowp.tile([128, L], BF16, tag="rs_bc")
        mrs_bc = rowp.tile([128, L], BF16, tag="mrs_bc")
        # token-major stat columns [128, 16]
        ncol = rowp.tile([128, NMT], F32, tag="ncol")
        sqm = rowp.tile([128, NMT], F32, tag="sqm")
        rs_cols = rowp.tile([128, NMT], F32, tag="rs_cols")
        nmrsc = rowp.tile([128, NMT], F32, tag="nmrsc")
        nmu_cols = ncol

        if not b1_zero:
            bvbc = cp.tile([128, DL], BF16, tag="bvbc")
            nc.sync.dma_start(bvbc[:], bvbc_d[:, :])

        # ---- stage A: x/wv DMA + column stats (ap=1 matmuls, ~free on PE)
        # + first v-chain group kt-major to fill PE during the DMA window ----
        NG1 = 5
        vmm_cm = tc.tile_pool(name="ps_vm", bufs=6, space="PSUM")
        vmm = vmm_cm.__enter__()
        vps_g1 = [vmm.tile([128, 512], F32, tag="vmm", name=f"vps{m}") for m in range(NG1)]
        with (
            tc.tile_pool(name="ps_sc", bufs=1, space="PSUM") as pscol,
        ):
            stco = pscol.tile([128, 2 * NMT], F32, tag="stco", name="stco")
            scol = stco[:, 0:NMT]
            sqcol = stco[:, NMT : 2 * NMT]
            for kt in range(KT):
                xc = xbp.tile([128, L], BF16, tag=f"xb{kt}", name=f"xb{kt}")
                if kt == 0:  # split so the first stats matmuls start sooner
                    for q4 in range(4):
                        qsl = slice(q4 * 512, (q4 + 1) * 512)
                        nc.sync.dma_start(xc[:, qsl], xt_d[0:128, qsl])
                else:
                    nc.sync.dma_start(xc[:], xt_d[kt * 128 : (kt + 1) * 128, :])
                xb.append(xc)
                wvt = wscp.tile([128, DL], BF16, tag=f"wv{kt}", name=f"wv{kt}")
                nc.sync.dma_start(wvt[:], wv_d[kt * 128 : (kt + 1) * 128, :])
                wsc.setdefault("v", []).append(wvt)
                xsq = kS[0]  # borrowed as x^2 staging until stage C smears
                if kt % 2 == 0:
                    nc.scalar.activation(xsq[:], xc[:], AF.Square)
                else:  # bf16 TT runs 2x on DVE; keeps ACT off the stats path
                    nc.vector.tensor_mul(xsq[:], xc[:], xc[:])
                # NOTE: start=True resets the WHOLE psum bank, so only the
                # very first matmul touching each bank may carry it.
                for m in range(NMT):
                    msl = slice(m * 128, (m + 1) * 128)
                    nc.tensor.matmul(
                        scol[:, m : m + 1], xc[:, msl], ones_bf[:],
                        start=(kt == 0 and m == 0), stop=(kt == KT - 1),
                        skip_group_check=True,
                    )
                    nc.tensor.matmul(
                        sqcol[:, m : m + 1], xsq[:, msl], ones_bf[:],
                        start=False, stop=(kt == KT - 1),
                        skip_group_check=True,
                    )
                for m in range(NG1):
                    nc.tensor.matmul(
                        vps_g1[m][:], xc[:, m * 128 : (m + 1) * 128], wvt[:],
                        start=(kt == 0), stop=(kt == KT - 1),
                    )
            emit_const_dmas()
            for kind, wd in (("q", wq_d), ("k", wk_d), ("p", wp_d)):
                t = wscp.tile([128, KT, 2, DL], E4, tag=f"w{kind}", name=f"w{kind}")
                nc.sync.dma_start(t[:, :, :, :], wd[:, :])
                wsc[kind] = t

            # column-space LN1 stats: all ops on [128, 16]
            nc.vector.tensor_scalar_mul(ncol[:], scol[:], -1.0 / D)
            nc.vector.tensor_scalar_mul(sqm[:], sqcol[:], 1.0 / D)
            nc.vector.tensor_mul(nmrsc[:], ncol[:], ncol[:])
            nc.vector.tensor_sub(sqm[:], sqm[:], nmrsc[:])  # var
            nc.scalar.activation(sqm[:], sqm[:], AF.Sqrt, bias=eps128[:])
            nc.vector.reciprocal(rs_cols[:], sqm[:])
            nc.vector.tensor_mul(nmrsc[:], ncol[:], rs_cols[:])

        # ---- stage B: broadcasts, remaining v chains, centering ----
        def emit_vpost(m, vps):
            # scale-copy first (frees the psum bank), then the token-major
            # -mu*rstd*colsum(Wv) correction runs all-SBUF on the idle Pool
            vt = resid.tile([128, DL], BF16, tag=f"vtok{m}", name=f"vtok{m}")
            nc.scalar.activation(vt[:], vps[:], AF.Copy, scale=rs_cols[:, m : m + 1])
            with nc.allow_low_precision("bf16 smear-v correction"):
                nc.vector.scalar_tensor_tensor(
                    vt[:], vcb[:], nmrsc[:, m : m + 1], vt[:], ALU.mult, ALU.add
                )
                if not b1_zero:  # beta term applied post-scale: * rstd_tok
                    nc.vector.scalar_tensor_tensor(
                        vt[:], bvbc[:], rs_cols[:, m : m + 1], vt[:],
                        ALU.mult, ALU.add,
                    )
            vtok.append(vt)

        def emit_vchain(m):
            msl = slice(m * 128, (m + 1) * 128)
            vps = vmm.tile([128, 512], F32, tag="vmm", name=f"vps{m}")
            for kt in range(KT):
                nc.tensor.matmul(vps[:], xb[kt][:, msl], wsc["v"][kt][:],
                                 start=(kt == 0), stop=(kt == KT - 1))
            return vps

        def gen_centering(ch):
            # xn-slice = x * rs_bc + mrs_bc on DVE (bf16 SBUF, 2x), then split
            # into fp8 hi (DVE copy, 2x_2p) + lo residual (Pool sub)
            csl = slice(ch * 512, (ch + 1) * 512)
            for kt in range(KT):
                xnc = xnr.tile([128, 512], BF16, tag="xnc", name=f"xnc{ch}_{kt}")
                nc.vector.tensor_mul(xnc[:], xb[kt][:, csl], rs_bc[:, csl])
                nc.vector.tensor_add(xnc[:], xnc[:], mrs_bc[:, csl])
                with nc.allow_low_precision("fp8 hi/lo split of xn"):
                    nc.vector.tensor_copy(xnq[:, kt, 1, csl], xnc[:])
                    nc.gpsimd.tensor_sub(xnq[:, kt, 0, csl], xnc[:], xnq[:, kt, 1, csl])
                yield

        def emit_centering(ch):
            for _ in gen_centering(ch):
                pass

        def weave(a, b, bsteps_per_astep):
            # merge two emission generators: advance b by ratio per a-step
            carry = 0.0
            bdone = False
            for _ in a:
                yield
                carry += bsteps_per_astep
                while carry >= 1.0 and not bdone:
                    carry -= 1.0
                    try:
                        next(b)
                    except StopIteration:
                        bdone = True
            while not bdone:
                try:
                    next(b)
                except StopIteration:
                    bdone = True
                else:
                    yield

        # m5 takes the spare psum slot with no wait; vposts free G1 slots
        vps_pend = [(NG1, emit_vchain(NG1))]
        for m in range(3):
            emit_vpost(m, vps_g1[m])
        vps_pend.append((NG1 + 1, emit_vchain(NG1 + 1)))
        vps_pend.append((NG1 + 2, emit_vchain(NG1 + 2)))

        # rstd / (-mu*rstd) broadcasts: diag(cols) matmul against all-ones;
        # diag construction rides the idle Pool engine
        with (
            tc.tile_pool(name="pbc", bufs=2, space="PSUM") as pbc,
            tc.tile_pool(name="dgp", bufs=8) as dgp,
        ):
            for ch in range(NCH):
                sl = slice(ch * 512, (ch + 1) * 512)
                for si, (src, dst) in enumerate(((rs_cols, rs_bc), (nmrsc, mrs_bc))):
                    bps = pbc.tile([128, 512], F32, tag="bcps", name=f"bc{si}_{ch}")
                    for mi in range(4):
                        m = 4 * ch + mi
                        dg = dgp.tile([128, 128], BF16, tag="dg")
                        nc.vector.tensor_scalar_mul(dg[:], idnb[:], src[:, m : m + 1])
                        nc.tensor.matmul(bps[:, mi * 128 : (mi + 1) * 128],
                                         ones128b[:], dg[:], start=(mi == 0),
                                         stop=(mi == 3), skip_group_check=True)
                    nc.scalar.copy(dst[:, sl], bps[:])

        for m in range(3, NG1):
            emit_vpost(m, vps_g1[m])
        emit_centering(0)  # qkp consumes ch0 first
        for m in range(NG1 + 3, NMT):
            vps_pend.append((m, emit_vchain(m)))
            if len(vps_pend) >= 3:
                emit_vpost(*vps_pend.pop(0))
            if m == NG1 + 4:
                emit_centering(1)
        while vps_pend:
            emit_vpost(*vps_pend.pop(0))
        # centering of chunks 2/3 is woven into the attention phases below so
        # the DVE queue serves attention tails between conversion batches

        vmm_cm.__exit__(None, None, None)
        pmm_cm = tc.tile_pool(name="ps_mm", bufs=3, space="PSUM")
        pmm = pmm_cm.__enter__()

        woutT = []
        for h in range(NH):
            t = wop.tile([128, D], BF16, tag=f"woutT{h}", name=f"woutT{h}")
            woutT.append(t)
        if not ln2_trivial:
            g2bc = wop.tile([128, D], F32, tag="g2bc")
            b2bc = wop.tile([128, D], F32, tag="b2bc")

        # ---- stages C-E: chunk-pipelined qkp + attention; out_proj on ch3 ----
        if True:

            def emit_qkp_mm(ps, wt, hsl, csl):
                # 3-term compensated fp8: hi@hi over kt-pairs, then the
                # (Whi,Wlo)x(xlo,xhi) cross terms per kt; 12 DoubleRow instrs
                # replace 8 bf16 instrs at 0.5 cyc/row -> 0.75x PE time
                for ktp in range(KT // 2):
                    nc.tensor.matmul(
                        ps, wt[:, 2 * ktp : 2 * ktp + 2, 0, hsl],
                        xnq[:, 2 * ktp : 2 * ktp + 2, 1, csl],
                        start=(ktp == 0), stop=False, perf_mode=PM.DoubleRow,
                    )
                for kt in range(KT):
                    nc.tensor.matmul(
                        ps, wt[:, kt, 0:2, hsl], xnq[:, kt, 0:2, csl],
                        start=False, stop=(kt == KT - 1), perf_mode=PM.DoubleRow,
                    )

            # rolling k scratch: col0 = previous chunk's last key, 1:513 = the
            # current chunk (saves 12KB/part vs per-head [128, L] buffers)
            kk = [kkp.tile([128, 513], BF16, tag=f"kk{h}", name=f"kk{h}") for h in range(NH)]

            def gen_qkp_ch(ch):
                # chunk-major: all heads consume chunk ch while ch+1 converts
                csl = slice(ch * 512, (ch + 1) * 512)
                for h in range(NH):
                    hsl = slice(h * 128, (h + 1) * 128)
                    qps = pmm.tile([128, 512], F32, tag="mm", name=f"qps{h}_{ch}")
                    emit_qkp_mm(qps[:], wsc["q"], hsl, csl)
                    nc.scalar.activation(qT[h][:, csl], qps[:], AF.Identity,
                                         bias=bqi[h], scale=inv_bc[h])
                    yield

                    kps = pmm.tile([128, 512], F32, tag="mm", name=f"kps{h}_{ch}")
                    if ch > 0:  # save prev chunk's last key before overwrite
                        nc.gpsimd.tensor_copy(kk[h][:, 0:1], kk[h][:, 512:513])
                    emit_qkp_mm(kps[:], wsc["k"], hsl, csl)
                    nc.scalar.activation(kk[h][:, 1:513], kps[:], AF.Identity,
                                         bias=bko[h], scale=om_bc[h])
                    yield

                    pps = pmm.tile([128, 512], F32, tag="mm", name=f"pps{h}_{ch}")
                    emit_qkp_mm(pps[:], wsc["p"], hsl, csl)
                    nc.scalar.activation(geff[h][:, csl], pps[:], AF.Silu,
                                         bias=bp[h], scale=2.0 ** -swp)
                    # smear per-chunk so attention can start behind the k ACTs
                    cs, ce = ch * 512, (ch + 1) * 512
                    if ch == 0:
                        nc.vector.tensor_copy(kS[h][:, 0:1], kk[h][:, 1:2])
                        nc.vector.scalar_tensor_tensor(
                            kS[h][:, 1:512], kk[h][:, 1:512], ratio_bc[h],
                            kk[h][:, 2:513], ALU.mult, ALU.add,
                        )
                    else:
                        nc.vector.scalar_tensor_tensor(
                            kS[h][:, cs:ce], kk[h][:, 0:512], ratio_bc[h],
                            kk[h][:, 1:513], ALU.mult, ALU.add,
                        )
                    yield

            # qkp(0) runs here, with the remaining centering conversions woven
            # through it so smears and conversions alternate on the DVE queue;
            # then the raw-x pools pop and their SBUF is reused for the
            # attention scratch pools below.
            def _chain(*gens):
                for g in gens:
                    yield from g

            for _ in gen_qkp_ch(0):
                pass
            emit_centering(2)
            emit_centering(3)
            xnr_cm.__exit__(None, None, None)
            xbp_cm.__exit__(None, None, None)

            pss_cm = tc.tile_pool(name="ps_s", bufs=2, space="PSUM")
            pss = pss_cm.__enter__()
            pso_cm = tc.tile_pool(name="ps_o", bufs=2, space="PSUM")
            pso = pso_cm.__enter__()
            psd_cm = tc.tile_pool(name="ps_den", bufs=1, space="PSUM")
            psd = psd_cm.__enter__()
            etp_cm = tc.tile_pool(name="et", bufs=14)
            etp = etp_cm.__enter__()
            dnp_cm = tc.tile_pool(name="dn", bufs=4)
            dnp = dnp_cm.__enter__()
            lnp_cm = tc.tile_pool(name="ln2", bufs=2)
            lnp = lnp_cm.__enter__()
            osp_cm = tc.tile_pool(name="ostage", bufs=4)
            osp = osp_cm.__enter__()

            def gen_attn(h, chs=None):
                hsl = slice(h * 128, (h + 1) * 128)
                for ch in (range(NCH) if chs is None else chs):
                    csl = slice(ch * 512, (ch + 1) * 512)
                    kb_lo = max(0, 4 * ch + 1 - WB[h])
                    kb_hi = 4 * ch + 3
                    ops_ps = pso.tile([128, 512], F32, tag="ops", name=f"ops{h}_{ch}")
                    dsc = psd.tile([128, 512], F32, tag="den", name=f"den{h}_{ch}")
                    den_ps = dsc[:, 0:4]
                    first = {qs: None for qs in range(4)}
                    state = {"den_started": False}

                    def emit_ops(kb, qs0, qs1, esl, et):
                        st = all(first[qs] is None for qs in range(qs0, qs1))
                        for qs in range(qs0, qs1):
                            if first[qs] is None:
                                first[qs] = kb
                        nc.tensor.matmul(
                            ops_ps[:, esl], vtok[kb][:, hsl], et[:, esl],
                            start=st, stop=(kb == kb_hi),
                        )
                        for qs in range(qs0, qs1):
                            qsl = slice(qs * 128, (qs + 1) * 128)
                            nc.tensor.matmul(
                                den_ps[:, qs : qs + 1], et[:, qsl], ones_bf[:],
                                start=(not state["den_started"]),
                                stop=(kb == 4 * ch + qs),
                                skip_group_check=True,
                            )
                            state["den_started"] = True

                    # software-pipeline by two kb: S(k+2)/exp(k+2) issue before
                    # ops(k), so PE never blocks in-order on exp latencies
                    pend_ops = []
                    for kb in range(kb_lo, kb_hi + 1):
                        qs0 = max(0, kb - 4 * ch)
                        qs1 = min(4, kb - 4 * ch + WB[h])
                        if qs0 >= qs1:
                            continue
                        nsl = slice(csl.start + qs0 * 128, csl.start + qs1 * 128)
                        esl = slice(qs0 * 128, qs1 * 128)
                        sps = pss.tile([128, 512], F32, tag="sps", name=f"sps{h}_{ch}_{kb}")
                        dqs = kb - 4 * ch  # in-span diagonal block, if any
                        if qs0 <= dqs < qs1:
                            # causal mask folded into PSUM: I @ (-1e10 upper)
                            # seeds the bank so exp() emits exact zeros above
                            # the diagonal -- no DVE mask multiply needed
                            nc.tensor.matmul(
                                sps[:, dqs * 128 : (dqs + 1) * 128], idnb[:],
                                tri[:], start=True, stop=False,
                                skip_group_check=True,
                            )
                            nc.tensor.matmul(
                                sps[:, esl], kS[h][:, kb * 128 : (kb + 1) * 128],
                                qT[h][:, nsl], start=False, stop=True,
                                skip_group_check=True,
                            )
                        else:
                            nc.tensor.matmul(
                                sps[:, esl], kS[h][:, kb * 128 : (kb + 1) * 128],
                                qT[h][:, nsl], start=True, stop=True,
                            )
                        et = etp.tile([128, 512], BF16, tag="et")
                        if h == 0:
                            for qs in range(qs0, qs1):
                                qsl = slice(qs * 128, (qs + 1) * 128)
                                dd = (4 * ch + qs) - kb
                                nc.scalar.activation(
                                    et[:, qsl], sps[:, qsl], AF.Exp,
                                    bias=bias_v[h][dd],
                                )
                        elif h == 1:
                            nc.scalar.activation(
                                et[:, esl], sps[:, esl], AF.Exp,
                                bias=bias_w[4 * ch - kb + 3],
                            )
                        else:
                            nc.scalar.activation(et[:, esl], sps[:, esl], AF.Exp)
                        yield
                        if len(pend_ops) >= 2:
                            emit_ops(*pend_ops.pop(0))
                        pend_ops.append((kb, qs0, qs1, esl, et))
                        yield
                    for p in pend_ops:
                        emit_ops(*p)
                    den_sb = dnp.tile([128, 4], BF16, tag="densb")
                    with nc.allow_low_precision("bf16 den feeds transpose"):
                        nc.vector.tensor_copy(den_sb[:], den_ps[:])
                    dr_ps = dsc[0:1, 64:320].bitcast(BF16)
                    for qs in range(4):
                        nc.tensor.matmul(
                            dr_ps[0:1, qs * 128 : (qs + 1) * 128],
                            den_sb[:, qs : qs + 1], idnb[:],
                            start=(qs == 0), stop=(qs == 3), is_transpose=True,
                            skip_group_check=True,
                        )
                    dinv = dnp.tile([1, 512], BF16, tag="dinv")
                    with nc.allow_low_precision("bf16 1/den feeds a bf16 matmul"):
                        nc.vector.reciprocal(dinv[:], dr_ps[:])
                    # dbc borrows the pso ring (ops slot of two units ago is free)
                    dbc_ps = pso.tile([128, 512], F32, tag="ops", name=f"dbc{h}_{ch}")
                    nc.tensor.matmul(dbc_ps[:], ones_bfr[:], dinv[:],
                                     start=True, stop=True)
                    dbc = dnp.tile([128, 512], BF16, tag="dbc")
                    with nc.allow_low_precision("bf16 1/den broadcast"):
                        nc.vector.tensor_copy(dbc[:], dbc_ps[:])
                    ozc = dnp.tile([128, 512], BF16, tag="ozc")
                    nc.vector.tensor_mul(ozc[:], ops_ps[:], dbc[:])
                    nc.vector.tensor_mul(geff[h][:, csl], ozc[:], geff[h][:, csl])
                    yield

            def emit_attn(h, chs=None):
                for _ in gen_attn(h, chs):
                    pass

            def interleave(main, bg, k):
                # advance bg ~k steps per main step; exhaust both
                carry = 0.0
                done = False
                for _ in main:
                    carry += k
                    while carry >= 1.0 and not done:
                        carry -= 1.0
                        try:
                            next(bg)
                        except StopIteration:
                            done = True
                while not done:
                    try:
                        next(bg)
                    except StopIteration:
                        done = True

            def outproj_chain(g, mi, nch2, eng_act):
                def emit():
                    m = 4 * g + mi
                    msl = slice(m * 128, (m + 1) * 128)
                    nsl2 = slice(nch2 * 512, (nch2 + 1) * 512)
                    op2 = pmm.tile([128, 512], F32, tag="mm", name=f"op2_{m}_{nch2}")
                    for hh in range(NH):
                        nc.tensor.matmul(
                            op2[:], geff[hh][:, msl], woutT[hh][:, nsl2],
                            start=(hh == 0), stop=(hh == NH - 1),
                        )
                    osb = osp.tile([128, 512], BF16, tag="osb")
                    with nc.allow_low_precision("bf16 residual staging"):
                        if eng_act:
                            nc.scalar.copy(osb[:], op2[:])
                        else:
                            nc.vector.tensor_copy(osb[:], op2[:])
                    nc.sync.dma_start(
                        rs_in[g][mi * 128 : (mi + 1) * 128, nsl2], osb[:]
                    )

                return emit

            def outproj_closures(g, act_frac=2):
                # act_frac: every act_frac-th copy goes to ACT (rest DVE);
                # the last groups run after the exps drain, so ACT is free
                return [
                    outproj_chain(g, mi, nch2,
                                  eng_act=((mi + nch2) % 2 == 0))
                    for mi in range(4)
                    for nch2 in range(2)
                ]

            def emit_outproj_fin(g):
                    if True:
                        if with_cc:
                            nc.gpsimd.collective_compute(
                                "ReduceScatter", ALU.add,
                                replica_groups=[[0, 1, 2, 3], [4, 5, 6, 7]],
                                ins=[rs_in[g][:, :].opt()],
                                outs=[rs_out[g][:, :].opt()],
                            )
                        else:
                            nc.sync.dma_start(rs_out[g][:, :], rs_in[g][0:128, :])
                        yt = lnp.tile([128, D], BF16, tag="yt")
                        nc.sync.dma_start(yt[:], rs_out[g][:, :])
                        bs = lnp.tile([128, 12], F32, tag="bs")
                        nc.vector.bn_stats(bs[:, 0:6], yt[:, 0:512])
                        nc.vector.bn_stats(bs[:, 6:12], yt[:, 512:1024])
                        ag = lnp.tile([128, 2], F32, tag="ag")
                        nc.vector.bn_aggr(ag[:], bs[:])
                        sd2 = lnp.tile([128, 1], F32, tag="sd2")
                        nc.scalar.activation(sd2[:], ag[:, 1:2], AF.Sqrt, bias=eps128[:])
                        rstd2 = lnp.tile([128, 1], F32, tag="rstd2")
                        nc.vector.reciprocal(rstd2[:], sd2[:])
                        nmu2 = lnp.tile([128, 1], F32, tag="nmu2")
                        nc.vector.scalar_tensor_tensor(
                            nmu2[:], ag[:, 0:1], -1.0, rstd2[:], ALU.mult, ALU.mult
                        )
                        t2 = lnp.tile([128, D], BF16, tag="t2")
                        nc.scalar.activation(t2[:], yt[:], AF.Identity, bias=nmu2[:], scale=rstd2[:])
                        if ln2_trivial:
                            nc.sync.dma_start(out_d[g * 128 : (g + 1) * 128, :], t2[:])
                        else:
                            t3 = lnp.tile([128, D], BF16, tag="t3")
                            nc.vector.tensor_mul(t3[:], t2[:], g2bc[:])
                            nc.vector.tensor_add(t3[:], t3[:], b2bc[:])
                            nc.sync.dma_start(out_d[g * 128 : (g + 1) * 128, :], t3[:])

            def gen_attn_ch(ch):
                for h in range(NH):
                    yield from gen_attn(h, [ch])

            def nkb(h, ch):
                kb_lo = max(0, 4 * ch + 1 - WB[h])
                n = 0
                for kb in range(kb_lo, 4 * ch + 4):
                    if max(0, kb - 4 * ch) < min(4, kb - 4 * ch + WB[h]):
                        n += 1
                return n

            def n_attn(ch):
                return sum(2 * nkb(h, ch) + 1 for h in range(NH))

            def gen_outproj_tail():
                for g in range(NCH - 1):
                    for c in outproj_closures(g):
                        c()
                        yield
                    emit_outproj_fin(g)
                    yield

            # software-pipelined emission: attention of chunk ch rides inside
            # the qkp GEMM phase of chunk ch+1 so exps hide under matmul cover
            interleave(gen_qkp_ch(1), gen_attn_ch(0), k=n_attn(0) / 12.0)
            interleave(gen_qkp_ch(2), gen_attn_ch(1), k=n_attn(1) / 12.0)
            for h in range(NH):
                nc.sync.dma_start(woutT[h][:], wout_d[h * 128 : (h + 1) * 128, :])
            if not ln2_trivial:
                nc.sync.dma_start(g2bc[:], g2bc_d[:, :])
                nc.sync.dma_start(b2bc[:], b2bc_d[:, :])
            interleave(gen_qkp_ch(3), gen_attn_ch(2), k=n_attn(2) / 12.0)
            # tail: out_proj of chunks 0..2 rides inside attention of chunk 3
            interleave(gen_attn_ch(3), gen_outproj_tail(),
                       k=(8 * (NCH - 1) + (NCH - 1)) / float(n_attn(3)))
            for c in outproj_closures(NCH - 1):
                c()
            emit_outproj_fin(NCH - 1)

            osp_cm.__exit__(None, None, None)
            lnp_cm.__exit__(None, None, None)
            dnp_cm.__exit__(None, None, None)
            etp_cm.__exit__(None, None, None)
            psd_cm.__exit__(None, None, None)
            pso_cm.__exit__(None, None, None)
            pss_cm.__exit__(None, None, None)

        pmm_cm.__exit__(None, None, None)
        kkp_cm.__exit__(None, None, None)
        wop_cm.__exit__(None, None, None)
        wscp_cm.__exit__(None, None, None)
        xnp_cm.__exit__(None, None, None)
        rowp_cm.__exit__(None, None, None)
        qkpA_cm.__exit__(None, None, None)
        dram_cm.__exit__(None, None, None)
        resid_cm.__exit__(None, None, None)
        cp_cm.__exit__(None, None, None)

    _normalize_waits(nc)
    return nc


def _slopes16():
    half = NHEADS // 2
    return np.concatenate(
        [2.0 ** np.linspace(0.0, -8.0, half), np.zeros(NHEADS - half)]
    ).astype(np.float32)


def kernel(x, ln1_g, ln1_b, ln2_g, ln2_b, w_in, w_out, smear_factor, log_scale):
    x = np.asarray(x, np.float32)
    w_in = np.asarray(w_in, np.float32)
    w_out = np.asarray(w_out, np.float32)
    ln1_g = np.asarray(ln1_g, np.float32)
    ln1_b = np.asarray(ln1_b, np.float32)
    ln2_g = np.asarray(ln2_g, np.float32)
    ln2_b = np.asarray(ln2_b, np.float32)
    smear_factor = np.asarray(smear_factor, np.float32)
    log_scale = np.asarray(log_scale, np.float32)

    # fold ln1 gamma into w_in host-side
    wg = w_in * ln1_g[:, None]
    bw = ln1_b @ wg  # [4*DEXP] contribution of ln1 beta

    def sw_for(cols):
        m = np.abs(wg[:, cols]).max()
        return int(np.floor(np.log2(120.0 / max(m, 1e-30))))

    swq = sw_for(slice(0 * DEXP, 1 * DEXP))
    swk = sw_for(slice(1 * DEXP, 2 * DEXP))
    swp = sw_for(slice(3 * DEXP, 4 * DEXP))

    def pack_fp8(Wcols, sw):
        # [1024, DL] f32 -> [128, KT*2*DL] fp8 (hi plane 0, lo plane 1)
        Ws = Wcols * (2.0 ** sw)
        hi = Ws.astype(NP_E4)
        lo = (Ws - hi.astype(np.float32)).astype(NP_E4)
        arr = np.stack(
            [hi.reshape(KT, 128, DL), lo.reshape(KT, 128, DL)], axis=2
        )  # [KT, 128, 2, DL]
        return np.ascontiguousarray(arr.transpose(1, 0, 2, 3)).reshape(128, -1)

    b1_zero = not np.any(ln1_b)
    ln2_trivial = (not np.any(ln2_b)) and np.all(ln2_g == 1.0)
    key = ("nc", b1_zero, ln2_trivial, swp)
    if key not in _CACHED:
        _CACHED[key] = build(b1_zero=b1_zero, ln2_trivial=ln2_trivial, swp=swp)
    nc = _CACHED[key]

    slopes16 = _slopes16()
    jj = np.arange(128)
    # "tri" slot now carries the additive causal mask: -1e10 above the
    # diagonal (j > i), 0 elsewhere; injected into score PSUM via I @ mask
    tri = np.where(jj[:, None] > jj[None, :], np.float32(-1e10),
                   np.float32(0.0)).astype(NP_BF16)
    idn = np.eye(128, dtype=np.float32)  # bf16 identity for PE transposes

    in_maps = []
    for c in range(8):
        b, r = divmod(c, 4)
        hs = HGROUPS[r]
        cols = np.concatenate([np.arange(h * 128, (h + 1) * 128) for h in hs])
        sl = slopes16[hs]
        inv = np.exp(-2.0 * log_scale[hs]) / np.sqrt(128.0)
        sg = 1.0 / (1.0 + np.exp(-smear_factor[hs]))
        om = 1.0 - sg
        ratio = np.exp(smear_factor[hs])
        # fold the fp8 weight pre-scales out through the ACT post-op scales
        hbc = np.tile(
            np.concatenate(
                [inv * 2.0 ** -swq, om * 2.0 ** -swk, ratio]
            ).reshape(1, 3 * NH),
            (128, 1),
        ).astype(np.float32)
        # per-head ln1-beta bias columns: q scaled by inv, k by om, p raw
        bq = bw[0 * DEXP + cols].reshape(NH, 128)
        bk = bw[1 * DEXP + cols].reshape(NH, 128)
        bpv = bw[3 * DEXP + cols].reshape(NH, 128)
        qkb = np.concatenate(
            [bq.T * inv[None, :], bk.T * om[None, :], bpv.T], axis=1
        ).astype(np.float32)  # [128, 12]
        wv_sl = np.ascontiguousarray(wg[:, 2 * DEXP + cols]).astype(np.float32)
        vcb = np.tile(wv_sl.sum(axis=0, dtype=np.float64).astype(np.float32)[None, :], (128, 1))
        iota_c = np.arange(128, dtype=np.float32)
        bias_cols = [sl[0] * (iota_c - 128 * d - 63) for d in range(NB0)]
        # slot1: one vector per dd = 4*ch - kb in [-3, 15]:
        bias_cols += [sl[1] * (iota_c - 128 * d - 447) for d in range(-3, 16)]
        biasv = np.stack(bias_cols, axis=1).astype(np.float32)
        m = {
            "xt": np.ascontiguousarray(x[b].T).astype(NP_BF16),
            "wq": pack_fp8(wg[:, 0 * DEXP + cols], swq),
            "wk": pack_fp8(wg[:, 1 * DEXP + cols], swk),
            "wv": wv_sl.astype(NP_BF16),
            "wp": pack_fp8(wg[:, 3 * DEXP + cols], swp),
            "wout": np.ascontiguousarray(w_out[cols, :]).astype(NP_BF16),
            "hbc": hbc,
            "qkb": qkb,
            "vcb": vcb.astype(NP_BF16),
            "biasv": biasv,
            "tri": tri,
            "idnb": idn.astype(NP_BF16),
        }
        if not b1_zero:
            m["bvbc"] = np.tile(bw[2 * DEXP + cols][None, :], (128, 1)).astype(NP_BF16)
        if not ln2_trivial:
            m["g2bc"] = np.tile(ln2_g.reshape(1, D), (128, 1)).astype(np.float32)
            m["b2bc"] = np.tile(ln2_b.reshape(1, D), (128, 1)).astype(np.float32)
        in_maps.append(m)

    res = None
    last_exc = None
    for _attempt in range(3):
        try:
            res = run_bass_kernel_spmd(nc, in_maps, core_ids=list(range(8)))
            break
        except Exception as e:  # transient axon worker drops; retry
            last_exc = e
            import time as _time

            _time.sleep(2.0)
    if res is None:
        raise last_exc
    _CACHED["last_res"] = res
    out = np.empty((B, L, D), np.float32)
    for c in range(8):
        b, r = divmod(c, 4)
        o = np.asarray(res.results[c]["out"], np.float32)  # [512, 1024] bf16->f32
        for g in range(NG):
            out[b, 512 * g + 128 * r : 512 * g + 128 * r + 128, :] = o[
                128 * g : 128 * (g + 1), :
            ]
    return out

